# revision 1
# baseline (speedup 1.0000x reference)
"""ASSAAttention (alpha1*relu(s)^2 + alpha2*softmax(s) blend) on 8 TRN2 cores.

Sharding: 32 (b,h) pairs -> 4 per core (SPMD, one NEFF, per-core input
slices). alpha1/alpha2 are computed on the host and baked as immediates.

Per-core pipeline (scores kept TRANSPOSED the whole way -- this avoids ever
transposing the 2048x2048 score matrix):
  - Q^T / K^T built once per (b,h) by PE transposes. K^T is PAIR-PACKED
    (s-blocks 2j / 2j+1 on partitions 0:64 / 64:128) and Q^T duplicated to
    both partition halves via a SBUF->SBUF DMA, so each QK step issues two
    row-packed fp32r matmuls (tile_position (0,0)/(64,0)) that share the
    128x128 PE array (K=64 would otherwise idle half the array).
  - scoresT pair [128, 2, 512] lands in PSUM; elementwise ops run
    double-width [128, 1024]:
      ACT: E = exp(s/8) -> fp16    (+1/4 of the relus)
      DVE: rl = max(s, 0) -> fp16, r2 = rl*rl (fp16, 2x mode)
  - PV accumulation over the 16 s-chunks (contraction = s on partitions):
      ssaT += (alpha1/64 * V)^T @ r2   [64, LB]  (fp16)
      dsaT += [alpha2*V | 1]^T @ E     [65, LB]  (fp32r; row 64 accumulates
              the softmax denominator for free)
  - L-block tail: accs -> SBUF, PE-transpose [64/65,128] tiles back,
    out = ssa_t + dsa_t * (1/sum) (reciprocal + scale + add), DMA out.
  - Next-(b,h) prep (DMAs, V prep on GPSIMD, Q/K transposes) is interleaved
    into the current (b,h)'s chunk stream to hide the boundary.
All matmuls use float32r: 1 cycle/row at N=512 with ~1.5e-4 relative error
(TF32-class); overall kernel error vs fp64 reference ~4e-4 L2.
Cost-model (TimelineSim) per-core time: ~264 us.
"""

import os
import sys

for _p in ("/opt/trn_rl_repo", "/root/.axon_site/_ro/trn_rl_repo"):
    if os.path.isdir(_p) and _p not in sys.path:
        sys.path.append(_p)

import numpy as np

import concourse.bass as bass  # noqa: F401  (bass types used via tile/bacc)
import concourse.tile as tile
from concourse import bacc, mybir
from concourse.tile_rust import add_dep_helper
from concourse.bass_utils import run_bass_kernel_spmd
from concourse.masks import make_identity

F32 = mybir.dt.float32
F32R = mybir.dt.float32r
FP16 = mybir.dt.float16
AF = mybir.ActivationFunctionType
ALU = mybir.AluOpType

N_CORES = 8
E = 64  # head dim
RELU_ACT_MOD = int(os.environ.get("RELU_ACT_MOD", "4"))  # 1/4 of relus on ACT
SQ_GPS_MOD = int(os.environ.get("SQ_GPS_MOD", "1000000"))  # squares stay on DVE
PAIR_WIDE = int(os.environ.get("PAIR_WIDE", "1"))  # fused double-width elemwise
SQ_ACT_MOD = int(os.environ.get("SQ_ACT_MOD", "1000000"))  # 1/N squares on ACT
BLEND_SCALE_DVE = int(os.environ.get("BLEND_SCALE_DVE", "0"))
BLEND_STT = int(os.environ.get("BLEND_STT", "0"))
DEP_SQ_RELU = int(os.environ.get("DEP_SQ_RELU", "0"))
RELU_ACT_J = int(os.environ.get("RELU_ACT_J", "-1"))  # >=0: first J pairs/blk on ACT
E_FP16 = int(os.environ.get("E_FP16", "1"))  # exp path in fp16 (separate LDW on HW)
SC_BUFS = int(os.environ.get("SC_BUFS", "2"))
BLP_BUFS = int(os.environ.get("BLP_BUFS", "2"))  # 0 -> share sc tags


def build_kernel(nbh, L, S, alpha1, alpha2, n_devices=N_CORES):
    """Build the per-core SPMD program. Returns a compiled Bacc."""
    assert L % 1024 == 0 or L in (256, 512)
    LB = int(os.environ.get("KLB", "0")) or min(L, 512)  # L-block size
    n_lb = L // LB
    SC = S // 128              # number of s-chunks
    NP = LB // 512 if LB >= 512 else 1   # 512-wide matmul pieces per block
    PW = min(LB, 512)          # matmul piece width
    n_t = LB // 128            # output L-tiles per block

    nc = bacc.Bacc("TRN2", target_bir_lowering=False, debug=False,
                   num_devices=n_devices)
    q_d = nc.dram_tensor("q", [nbh, L, E], F32, kind="ExternalInput").ap()
    k_d = nc.dram_tensor("k", [nbh, S, E], F32, kind="ExternalInput").ap()
    v_d = nc.dram_tensor("v", [nbh, S, E], F32, kind="ExternalInput").ap()
    o_d = nc.dram_tensor("o", [nbh, L, E], F32, kind="ExternalOutput").ap()

    qk_scale = 1.0 / np.sqrt(E)

    with tile.TileContext(nc) as tc:
        with (
            tc.tile_pool(name="const", bufs=1) as constp,
            tc.tile_pool(name="inp", bufs=2) as inp,
            tc.tile_pool(name="wt", bufs=2) as wt,
            tc.tile_pool(name="ew", bufs=int(os.environ.get("EW_BUFS","4"))) as ew,
            tc.tile_pool(name="rw", bufs=int(os.environ.get("RW_BUFS","4"))) as rw,
            tc.tile_pool(name="osb", bufs=int(os.environ.get("OSB_BUFS", "2"))) as osb,
            tc.tile_pool(name="tiny", bufs=int(os.environ.get("TINY_BUFS", "4"))) as tiny,
            tc.tile_pool(name="sc", bufs=SC_BUFS, space="PSUM") as scp,
            tc.tile_pool(name="acc", bufs=1, space="PSUM") as accp,
            tc.tile_pool(name="blp", bufs=max(BLP_BUFS, 1), space="PSUM") as blp,
        ):
            ident = constp.tile([128, 128], F32, tag="ident")
            make_identity(nc, ident)

            LT = L // 128  # l-tiles
            ST = S // 128  # s-tiles

            def prep_steps(bh):
                """Emit-later thunks that load/transform inputs for `bh`.
                Returns (steps, handles); handles filled as steps run."""
                h = {}

                def dma_in():
                    # split per 4-tile group so downstream transposes can
                    # start as soon as the first 128KB lands
                    h["q_in"] = inp.tile([128, LT, E], F32, tag="qin", name="q_in")
                    h["k_in"] = inp.tile([128, ST, E], F32, tag="kin", name="k_in")
                    h["v_in"] = inp.tile([128, ST, E], F32, tag="vin", name="v_in")
                    kv = k_d[bh].rearrange("(i p) e -> p i e", p=128)
                    qv = q_d[bh].rearrange("(i p) e -> p i e", p=128)
                    vv = v_d[bh].rearrange("(i p) e -> p i e", p=128)
                    for g in range(0, max(ST, LT), 4):
                        if g < ST:
                            ge = min(g + 4, ST)
                            nc.sync.dma_start(out=h["k_in"][:, g:ge, :],
                                              in_=kv[:, g:ge, :])
                        if g < LT:
                            ge = min(g + 4, LT)
                            nc.sync.dma_start(out=h["q_in"][:, g:ge, :],
                                              in_=qv[:, g:ge, :])
                    for g in range(0, ST, 8):
                        ge = min(g + 8, ST)
                        nc.sync.dma_start(out=h["v_in"][:, g:ge, :],
                                          in_=vv[:, g:ge, :])

                def v_prep():
                    # dsa stationary: [alpha2 * V | 1] (fp32r); ssa stationary:
                    # alpha1/64 * V (fp16) -- relu^2 path carries raw scores,
                    # so qk_scale^2 * alpha1 folds in here.
                    vdt = FP16 if E_FP16 else F32R
                    h["v_aug"] = wt.tile([128, ST, E + 1], vdt, tag="vaug", name="v_aug")
                    nc.gpsimd.tensor_scalar(
                        out=h["v_aug"][:, :, E:E + 1], in0=h["v_in"][:, :, 0:1],
                        scalar1=0.0, scalar2=1.0, op0=ALU.mult, op1=ALU.add)
                    nc.gpsimd.tensor_scalar(
                        out=h["v_aug"][:, :, 0:E], in0=h["v_in"],
                        scalar1=float(alpha2), scalar2=None, op0=ALU.mult)
                    h["v_bf"] = wt.tile([128, ST, E], FP16, tag="vbf", name="v_bf")
                    nc.gpsimd.tensor_scalar(
                        out=h["v_bf"], in0=h["v_in"],
                        scalar1=float(alpha1 / E), scalar2=None, op0=ALU.mult)

                def alloc_t():
                    h["qt"] = wt.tile([128, L], F32R, tag="qt", name="qt")
                    h["kt"] = wt.tile([128, S // 2], F32R, tag="kt", name="kt")

                def tr_group_q(g):
                    # 4 q l-tiles -> qt2 top rows [0:64]; bottom half is a
                    # SBUF->SBUF DMA duplicate (keeps engines free)
                    gw = min(4, LT - g)
                    use_blp = LB <= 512 and BLP_BUFS > 0
                    trp = blp if use_blp else scp
                    tr = trp.tile([64, 512], F32,
                                  tag=("blp" if use_blp else "sc"))
                    for i in range(gw):
                        nc.tensor.transpose(
                            tr[:, i * 128:(i + 1) * 128],
                            h["q_in"][:, g + i, :], ident)
                    csl = slice(g * 128, (g + gw) * 128)
                    if bh == 0:
                        # startup: DVE is idle; don't serialize behind ACT
                        nc.vector.tensor_copy(h["qt"][0:64, csl],
                                              tr[:, 0:gw * 128])
                    else:
                        nc.scalar.activation(h["qt"][0:64, csl],
                                             tr[:, 0:gw * 128], AF.Copy)
                    nc.sync.dma_start(out=h["qt"][64:128, csl],
                                      in_=h["qt"][0:64, csl])

                def tr_group_k(g):
                    # 4 s-block PAIRS -> kt2 [128, 4*128]; pair 2j/2j+1
                    # lands on partitions 0:64 / 64:128 of column block j
                    gw = min(4, ST // 2 - g)
                    use_blp = LB <= 512 and BLP_BUFS > 0
                    trp = blp if use_blp else scp
                    tr = trp.tile([128, 512], F32,
                                  tag=("blp" if use_blp else "sc"))
                    for i in range(gw):
                        pair = h["k_in"][:, 2 * (g + i):2 * (g + i) + 2, :]
                        nc.tensor.transpose(
                            tr[:, i * 128:(i + 1) * 128],
                            pair.rearrange("p c e -> p (c e)"), ident)
                    nc.scalar.activation(
                        h["kt"][:, g * 128:(g + gw) * 128], tr[:, 0:gw * 128],
                        AF.Copy)

                steps = [dma_in, alloc_t, v_prep]
                # interleave k/q transpose groups to match DMA landing
                # order (k groups are issued first)
                kg = [lambda g=g: tr_group_k(g) for g in range(0, ST // 2, 4)]
                qg = [lambda g=g: tr_group_q(g) for g in range(0, LT, 4)]
                for i in range(max(len(kg), len(qg))):
                    if i < len(kg):
                        steps.append(kg[i])
                    if i < len(qg):
                        steps.append(qg[i])
                return steps, h

            def tail_copies(box, ssa_ps, dsa_ps):
                ssa_sb = osb.tile([64, LB], F32, tag="ssasb", name="ssa_sb")
                dsa_sb = osb.tile([E + 1, LB], F32, tag="dsasb", name="dsa_sb")
                nc.vector.tensor_copy(ssa_sb, ssa_ps)
                nc.scalar.activation(dsa_sb, dsa_ps, AF.Copy)
                box["ssa_sb"] = ssa_sb
                box["dsa_sb"] = dsa_sb

            def tail_blend(box, bh, lb):
                ssa_sb, dsa_sb = box["ssa_sb"], box["dsa_sb"]
                out_sb = osb.tile([128, n_t, E], F32, tag="outsb",
                                  name="out_sb")
                for t in range(n_t):
                    tsl = slice(t * 128, (t + 1) * 128)
                    use_blp = LB <= 512 and BLP_BUFS > 0
                    trp = blp if use_blp else scp
                    tr = trp.tile([128, 2 * E + 1], F32,
                                  tag=("blp" if use_blp else "sc"), name="tr")
                    nc.tensor.transpose(
                        tr[:, 0:E], ssa_sb[:, tsl], ident[0:64, 0:64])
                    nc.tensor.transpose(
                        tr[:, E:2 * E + 1], dsa_sb[:, tsl],
                        ident[0:E + 1, 0:E + 1])
                    rcp = tiny.tile([128, 1], F32, tag="rcp", name="rcp")
                    nc.vector.reciprocal(rcp, tr[:, 2 * E:2 * E + 1])
                    tmp = tiny.tile([128, E], F32, tag="tmp", name="tmp")
                    if BLEND_SCALE_DVE:
                        nc.vector.tensor_scalar(
                            out=tmp, in0=tr[:, E:2 * E], scalar1=rcp,
                            scalar2=None, op0=ALU.mult)
                    else:
                        nc.scalar.activation(tmp, tr[:, E:2 * E],
                                             AF.Copy, scale=rcp)
                    nc.vector.tensor_add(out_sb[:, t, :], tmp, tr[:, 0:E])
                nc.sync.dma_start(
                    out=o_d[bh, lb * LB:(lb + 1) * LB, :].rearrange(
                        "(t p) e -> p t e", p=128),
                    in_=out_sb)

            chunk_idx = 0  # global chunk counter for ACT/DVE relu balancing
            prev_sq = None  # last DVE square instruction (ordering hint)
            pending_tail = []  # deferred L-block tails (drain/blend/store)
            steps0, h0 = prep_steps(0)
            for st in steps0:
                st()
            cur = h0

            for bh in range(nbh):
                nxt_steps, nxt_h = prep_steps(bh + 1) if bh + 1 < nbh else ([], None)
                qt, kt = cur["qt"], cur["kt"]
                v_aug, v_bf = cur["v_aug"], cur["v_bf"]

                # ---- main loops ----
                for lb in range(n_lb):
                    ssa_ps = accp.tile([64, LB], F32, tag="accs")
                    dsa_ps = accp.tile([E + 1, LB], F32, tag="accd")
                    for j in range(SC // 2):
                        # row-packed QK: s-blocks 2j (PE rows 0-63) and 2j+1
                        # (rows 64-127) compute concurrently into one
                        # [128, 2, LB] psum tile; elementwise ops then run
                        # double-width (halves per-op overhead)
                        sc_t = scp.tile([128, 2, LB], F32, tag="sc", name="sc_t")
                        for c in range(NP):
                            cl = slice(c * PW, (c + 1) * PW)
                            ql = slice(lb * LB + c * PW,
                                       lb * LB + (c + 1) * PW)
                            nc.tensor.matmul(
                                sc_t[:, 0, cl], kt[0:64, j * 128:(j + 1) * 128],
                                qt[0:64, ql], start=True, stop=True,
                                tile_position=(0, 0))
                            nc.tensor.matmul(
                                sc_t[:, 1, cl], kt[64:128, j * 128:(j + 1) * 128],
                                qt[64:128, ql], start=True, stop=True,
                                tile_position=(64, 0))
                        e_t = ew.tile([128, 2, LB],
                                      FP16 if E_FP16 else F32R, tag="e")
                        rl = rw.tile([128, 2, LB], FP16, tag="rl")
                        r2 = rw.tile([128, 2, LB], FP16, tag="r2")
                        if PAIR_WIDE:
                            parts = [(slice(0, 2), "p c l -> p (c l)")]
                        else:
                            parts = [(slice(0, 1), "p c l -> p (c l)"),
                                     (slice(1, 2), "p c l -> p (c l)")]
                        for psl, rr in parts:
                            sc_w = sc_t[:, psl, :].rearrange(rr)
                            nc.scalar.activation(
                                e_t[:, psl, :].rearrange(rr), sc_w, AF.Exp,
                                scale=qk_scale)
                            rl_w = rl[:, psl, :].rearrange(rr)
                            if RELU_ACT_J >= 0:
                                relu_on_act = j < RELU_ACT_J
                            else:
                                relu_on_act = chunk_idx % RELU_ACT_MOD == 0
                            if relu_on_act:
                                nc.scalar.activation(rl_w, sc_w, AF.Relu)
                            else:
                                ri = nc.vector.tensor_scalar(
                                    out=rl_w, in0=sc_w,
                                    scalar1=0.0, scalar2=None, op0=ALU.max)
                                if prev_sq is not None and DEP_SQ_RELU:
                                    # keep DVE in relu->sq->relu order: the
                                    # scheduler otherwise hoists the next
                                    # relu (frees a psum slot) ahead of the
                                    # square, delaying this pair's PV
                                    add_dep_helper(
                                        ri.ins, prev_sq.ins, sync=False,
                                        reason="sq before next DVE relu")
                            r2_w = r2[:, psl, :].rearrange(rr)
                            if chunk_idx % SQ_GPS_MOD == 0:
                                nc.gpsimd.tensor_mul(r2_w, rl_w, rl_w)
                            elif chunk_idx % SQ_ACT_MOD == SQ_ACT_MOD - 1:
                                nc.scalar.activation(r2_w, rl_w, AF.Square)
                            else:
                                prev_sq = nc.vector.tensor_mul(r2_w, rl_w, rl_w)
                            chunk_idx += 1
                        if j < 2 and pending_tail:
                            pending_tail.pop(0)()
                        # PV accumulation (two s-blocks)
                        for pp in range(2):
                            s = 2 * j + pp
                            first = s == 0
                            last = s == SC - 1
                            for c in range(NP):
                                sl = slice(c * PW, (c + 1) * PW)
                                nc.tensor.matmul(
                                    ssa_ps[:, sl], v_bf[:, s, :], r2[:, pp, sl],
                                    start=first, stop=last)
                                nc.tensor.matmul(
                                    dsa_ps[:, sl], v_aug[:, s, :], e_t[:, pp, sl],
                                    start=first, stop=last)
                        # interleave next-bh prep into this bh's chunk stream
                        bh_chunk = lb * (SC // 2) + j
                        if nxt_steps and (
                                bh_chunk == (n_lb - 1) * (SC // 2) // 2
                                or lb == n_lb - 1):
                            nxt_steps.pop(0)()
                    # ---- defer the tail: emitted between the next
                    # block's first pairs so QK/exp never wait on it ----
                    box = {}
                    pending_tail.append(
                        lambda box=box, s=ssa_ps, d=dsa_ps: tail_copies(box, s, d))
                    pending_tail.append(
                        lambda box=box, bh=bh, lb=lb: tail_blend(box, bh, lb))
                # flush any remaining prep for the next bh
                for st in nxt_steps:
                    st()
                cur = nxt_h
            # flush the last L-block's tail
            for fn in pending_tail:
                fn()

    nc.compile()
    return nc


def execute(inputs, **run_kwargs):
    """Run the full problem; returns (output, BassKernelResults)."""
    queries = np.asarray(inputs["queries"], dtype=np.float32)
    keys = np.asarray(inputs["keys"], dtype=np.float32)
    values = np.asarray(inputs["values"], dtype=np.float32)
    a1 = float(np.asarray(inputs["a1"]))
    a2 = float(np.asarray(inputs["a2"]))

    B, L, H, Edim = queries.shape
    assert Edim == E
    w1, w2 = np.exp(a1), np.exp(a2)
    alpha1 = w1 / (w1 + w2)
    alpha2 = w2 / (w1 + w2)

    # [B, L, H, E] -> [B*H, L, E]
    qh = np.ascontiguousarray(queries.transpose(0, 2, 1, 3)).reshape(B * H, L, E)
    kh = np.ascontiguousarray(keys.transpose(0, 2, 1, 3)).reshape(B * H, L, E)
    vh = np.ascontiguousarray(values.transpose(0, 2, 1, 3)).reshape(B * H, L, E)

    nbh = (B * H) // N_CORES
    nc = build_kernel(nbh, L, L, alpha1, alpha2)

    in_maps = []
    for i in range(N_CORES):
        sl = slice(i * nbh, (i + 1) * nbh)
        in_maps.append({"q": qh[sl], "k": kh[sl], "v": vh[sl]})

    res = run_bass_kernel_spmd(nc, in_maps, core_ids=list(range(N_CORES)),
                               **run_kwargs)
    out = np.concatenate([r["o"] for r in res.results], axis=0)  # [B*H, L, E]
    out = out.reshape(B, H, L, E).transpose(0, 2, 1, 3)
    return np.ascontiguousarray(out), res


def kernel(**inputs):
    out, _ = execute(inputs)
    return out


if __name__ == "__main__":
    # tiny smoke test: single core, small shapes
    rng = np.random.default_rng(0)
    nbh, L = 1, 256
    q = rng.standard_normal((nbh, L, E), dtype=np.float32)
    k = rng.standard_normal((nbh, L, E), dtype=np.float32)
    v = rng.standard_normal((nbh, L, E), dtype=np.float32)
    a1 = a2 = 1.0
    nc = build_kernel(nbh, L, L, 0.5, 0.5, n_devices=1)
    res = run_bass_kernel_spmd(
        nc, [{"q": q, "k": k, "v": v}], core_ids=[0]).results[0]
    got = res["o"].astype(np.float64)

    # numpy reference
    s = np.einsum("ble,bse->bls", q, k).astype(np.float64) / np.sqrt(E)
    ssa = np.maximum(s, 0) ** 2
    dsa = np.exp(s - s.max(-1, keepdims=True))
    dsa /= dsa.sum(-1, keepdims=True)
    ref = 0.5 * np.einsum("bls,bse->ble", ssa, v) + \
        0.5 * np.einsum("bls,bse->ble", dsa, v)
    print("l2_rel:", np.linalg.norm(got - ref) / np.linalg.norm(ref))

